# revision 1
# baseline (speedup 1.0000x reference)
"""MoE gate (softmax routing, top-6 of 64 experts) for Trainium2, 8 NeuronCores.

Problem: x (4, 4096, 2048) f32, gate weight (64, 2048) f32.
  logits = x @ w.T          (16384, 64)
  scores = softmax(logits)
  topk_weight, topk_idx = top_k(scores, 6)       (sorted desc)
  aux_loss = seq-aux load-balancing loss (scalar)

Sharding: data-parallel over the flattened token dim — 2048 tokens per core.
Each core's shard is fed pre-transposed (C-major) so the contraction dim C
lands on SBUF partitions for the PE matmul. The gate weight (tiny) is
replicated. Per-core partial count/score sums are combined on the host into
the scalar aux loss.

Device pipeline per core (tokens processed in quarters of 512):
  DMA xT chunk-groups -> PE matmul (wT stationary, tokens moving) accumulating
  logitsT (64, 512) in PSUM over 16 c-chunks -> copy to SBUF -> PE transpose
  to (128 tok, 64 e) -> DVE max8/max_index (top-8 per token, desc order) ->
  ACT exp (no max-subtract: |logits| is O(1)) -> DVE reduce/reciprocal for
  softmax denom -> top-k weights, per-expert count + score partial sums.
"""

import os
import sys
import numpy as np
from contextlib import ExitStack

sys.path.insert(0, "/opt/trn_rl_repo")

import concourse.bass as bass
import concourse.bacc as bacc
import concourse.mybir as mybir
import concourse.tile as tile
from concourse import masks
from concourse import bass_utils

# ---- problem constants (hardcoded per the contract) ----
TOP_K = 6
E = 64               # experts
C = 2048             # feature dim
B, T = 4, 4096
N_TOKENS = B * T     # 16384
N_CORES = 8
TOK_PER_CORE = N_TOKENS // N_CORES   # 2048
N_CHUNKS = C // 128                  # 16 contraction chunks
Q = 4                                # quarters per core
TOK_Q = TOK_PER_CORE // Q            # 512 tokens per quarter
BLK_Q = TOK_Q // 128                 # 4 blocks of 128 tokens per quarter
G = 4                                # DMA chunk-groups per quarter
CPG = N_CHUNKS // G                  # 4 c-chunks per group
ALPHA = 0.001
ROUTED_SCALING = 1.0

F32 = mybir.dt.float32
U32 = mybir.dt.uint32


def build_nc(mm_dtype=mybir.dt.float32r):
    nc = bacc.Bacc("TRN2", target_bir_lowering=False, debug=False)

    xT = nc.dram_tensor("xT", (C, TOK_PER_CORE), F32, kind="ExternalInput")
    wT = nc.dram_tensor("wT", (C, E), F32, kind="ExternalInput")
    idx8 = nc.dram_tensor("idx8", (Q, BLK_Q, 128, 8), U32, kind="ExternalOutput")
    val8 = nc.dram_tensor("val8", (Q, BLK_Q, 128, 8), F32, kind="ExternalOutput")
    csum = nc.dram_tensor("csum", (Q, 128, E), F32, kind="ExternalOutput")
    ssum = nc.dram_tensor("ssum", (Q, 128, E), F32, kind="ExternalOutput")

    # (c, t) -> (p, k, t): chunk k, partition p = row k*128+p
    xT_v = xT.ap().rearrange("(k p) t -> p k t", p=128)
    wT_v = wT.ap().rearrange("(k p) e -> p k e", p=128)

    with tile.TileContext(nc) as tc, ExitStack() as ctx:
        const_pool = ctx.enter_context(tc.tile_pool(name="const", bufs=1))
        xpool = ctx.enter_context(tc.tile_pool(name="x", bufs=2))
        ps_mm = ctx.enter_context(tc.tile_pool(name="ps_mm", bufs=2, space="PSUM"))
        ps_tr = ctx.enter_context(tc.tile_pool(name="ps_tr", bufs=2, space="PSUM"))
        work = ctx.enter_context(tc.tile_pool(name="work", bufs=2))
        outp = ctx.enter_context(tc.tile_pool(name="outp", bufs=2))

        ident = const_pool.tile([128, 128], F32)
        masks.make_identity(nc, ident[:])

        wt = const_pool.tile([128, N_CHUNKS, E], F32)
        nc.sync.dma_start(wt[:], wT_v)

        for q in range(Q):
            tq = slice(q * TOK_Q, (q + 1) * TOK_Q)
            xg = []
            for g in range(G):
                t = xpool.tile([128, CPG, TOK_Q], F32, tag=f"xg{g}")
                nc.sync.dma_start(t[:], xT_v[:, g * CPG:(g + 1) * CPG, tq])
                xg.append(t)

            # logitsT (64, 512) accumulated over 16 c-chunks
            ps = ps_mm.tile([64, TOK_Q], F32)
            for k in range(N_CHUNKS):
                g, kk = divmod(k, CPG)
                nc.tensor.matmul(
                    ps[:],
                    wt[:, k, :].bitcast(mm_dtype),
                    xg[g][:, kk, :].bitcast(mm_dtype),
                    start=(k == 0),
                    stop=(k == N_CHUNKS - 1),
                )

            lt = work.tile([64, TOK_Q], F32, tag="lt")
            nc.vector.tensor_copy(lt[:], ps[:])

            # transpose to (128 tokens, 64 experts) per 128-token block
            pt = ps_tr.tile([128, BLK_Q, E], F32)
            for j in range(BLK_Q):
                nc.tensor.transpose(
                    pt[:, j, :], lt[:, j * 128:(j + 1) * 128], ident[:64, :64]
                )
            lg = work.tile([128, BLK_Q, E], F32, tag="lg")
            nc.vector.tensor_copy(lg[:], pt[:])

            # top-8 (desc) values + indices per token
            mx = work.tile([128, BLK_Q, 8], F32, tag="mx")
            ix = work.tile([128, BLK_Q, 8], U32, tag="ix")
            for j in range(BLK_Q):
                nc.vector.max(mx[:, j, :], lg[:, j, :])
                nc.vector.max_index(ix[:, j, :], mx[:, j, :], lg[:, j, :])

            # softmax pieces (no max subtraction; logits are O(1))
            eg = work.tile([128, BLK_Q, E], F32, tag="eg")
            nc.scalar.activation(eg[:], lg[:], mybir.ActivationFunctionType.Exp)
            dn = work.tile([128, BLK_Q], F32, tag="dn")
            nc.vector.reduce_sum(dn[:], eg[:], axis=mybir.AxisListType.X)
            rc = work.tile([128, BLK_Q], F32, tag="rc")
            nc.vector.reciprocal(rc[:], dn[:])

            e8 = work.tile([128, BLK_Q, 8], F32, tag="e8")
            nc.scalar.activation(e8[:], mx[:], mybir.ActivationFunctionType.Exp)
            w8 = outp.tile([128, BLK_Q, 8], F32, tag="w8")
            nc.vector.tensor_tensor(
                w8[:], e8[:], rc[:].unsqueeze(-1).broadcast_to((128, BLK_Q, 8)),
                op=mybir.AluOpType.mult,
            )

            # full scores + per-expert partial sums (over this quarter's tokens)
            sc = work.tile([128, BLK_Q, E], F32, tag="sc")
            nc.vector.tensor_tensor(
                sc[:], eg[:], rc[:].unsqueeze(-1).broadcast_to((128, BLK_Q, E)),
                op=mybir.AluOpType.mult,
            )
            sv = outp.tile([128, E], F32, tag="sv")
            nc.vector.reduce_sum(
                sv[:], sc[:].transpose((0, 2, 1)), axis=mybir.AxisListType.X
            )

            # count mask: logit >= 6th-largest  (exactly top-6 barring exact ties)
            mk = work.tile([128, BLK_Q, E], F32, tag="mk")
            nc.vector.tensor_tensor(
                mk[:], lg[:], mx[:, :, TOP_K - 1].unsqueeze(-1).broadcast_to((128, BLK_Q, E)),
                op=mybir.AluOpType.is_ge,
            )
            cv = outp.tile([128, E], F32, tag="cv")
            nc.vector.reduce_sum(
                cv[:], mk[:].transpose((0, 2, 1)), axis=mybir.AxisListType.X
            )

            # outputs: SBUF (p, j, k) -> DRAM (q, j, p, k)
            nc.sync.dma_start(idx8.ap()[q].transpose((1, 0, 2)), ix[:])
            nc.sync.dma_start(val8.ap()[q].transpose((1, 0, 2)), w8[:])
            nc.sync.dma_start(csum.ap()[q], cv[:])
            nc.sync.dma_start(ssum.ap()[q], sv[:])

    nc.compile()
    return nc, (xT, wT, idx8, val8, csum, ssum)


_NC_CACHE = {}


def _get_nc(mm_dtype_name):
    if mm_dtype_name not in _NC_CACHE:
        _NC_CACHE[mm_dtype_name] = build_nc(getattr(mybir.dt, mm_dtype_name))
    return _NC_CACHE[mm_dtype_name]


def _postprocess(results):
    """Combine per-core outputs into full (topk_idx, topk_weight, aux_loss)."""
    idx_parts, w_parts = [], []
    counts = np.zeros((N_CORES, E), np.float32)
    scores = np.zeros((N_CORES, E), np.float32)
    for c, out in enumerate(results):
        i8 = out["idx8"]          # (Q, BLK_Q, 128, 8)
        v8 = out["val8"]
        idx_parts.append(i8.reshape(TOK_PER_CORE, 8)[:, :TOP_K].astype(np.int32))
        w_parts.append(v8.reshape(TOK_PER_CORE, 8)[:, :TOP_K].astype(np.float32))
        counts[c] = out["csum"].sum(axis=(0, 1))
        scores[c] = out["ssum"].sum(axis=(0, 1))
    topk_idx = np.concatenate(idx_parts, axis=0)
    topk_weight = np.concatenate(w_parts, axis=0) * np.float32(ROUTED_SCALING)

    # aux loss: combine the two shards of each batch row
    ce = counts.reshape(B, 2, E).sum(axis=1) / (T * TOP_K / E)
    mean_scores = scores.reshape(B, 2, E).sum(axis=1) / T
    aux_loss = np.float32((ce * mean_scores).sum(axis=1).mean() * ALPHA)
    return topk_idx, topk_weight, aux_loss


def _make_in_maps(x, weight):
    xf = np.ascontiguousarray(np.asarray(x, dtype=np.float32).reshape(N_TOKENS, C))
    wT = np.ascontiguousarray(np.asarray(weight, dtype=np.float32).T)
    in_maps = []
    for c in range(N_CORES):
        shard = np.ascontiguousarray(xf[c * TOK_PER_CORE:(c + 1) * TOK_PER_CORE].T)
        in_maps.append({"xT": shard, "wT": wT})
    return in_maps


def kernel(x, weight, mm_dtype_name="float32r", trace=False):
    nc, _ = _get_nc(mm_dtype_name)
    in_maps = _make_in_maps(x, weight)
    res = bass_utils.run_bass_kernel_spmd(
        nc, in_maps, core_ids=list(range(N_CORES)), trace=trace
    )
    out = _postprocess(res.results)
    if trace:
        return out, res
    return out


# revision 3
# speedup vs baseline: 1.0209x; 1.0209x over previous
"""MoE gate (softmax routing, top-6 of 64 experts) for Trainium2, 8 NeuronCores.

Problem: x (4, 4096, 2048) f32, gate weight (64, 2048) f32.
  logits = x @ w.T          (16384, 64)
  scores = softmax(logits)
  topk_weight, topk_idx = top_k(scores, 6)       (sorted desc)
  aux_loss = seq-aux load-balancing loss (scalar)

Sharding: data-parallel over the flattened token dim — 2048 tokens per core.
Each core's shard is fed pre-transposed (C-major) so the contraction dim C
lands on SBUF partitions for the PE matmul. The gate weight (tiny) is
replicated. Per-core partial count/score sums are combined on the host into
the scalar aux loss.

Device pipeline per core (tokens processed in quarters of 512):
  DMA xT chunk-groups -> PE matmul (wT stationary, tokens moving) accumulating
  logitsT (64, 512) in PSUM over 16 c-chunks -> copy to SBUF -> PE transpose
  to (128 tok, 64 e) -> DVE max8/max_index (top-8 per token, desc order) ->
  ACT exp (no max-subtract: |logits| is O(1)) -> DVE reduce/reciprocal for
  softmax denom -> top-k weights, per-expert count + score partial sums.
"""

import os
import sys
import numpy as np
from contextlib import ExitStack

sys.path.insert(0, "/opt/trn_rl_repo")

import concourse.bass as bass
import concourse.bacc as bacc
import concourse.mybir as mybir
import concourse.tile as tile
from concourse import masks
from concourse import bass_utils

# ---- problem constants (hardcoded per the contract) ----
TOP_K = 6
E = 64               # experts
C = 2048             # feature dim
B, T = 4, 4096
N_TOKENS = B * T     # 16384
N_CORES = 8
TOK_PER_CORE = N_TOKENS // N_CORES   # 2048
N_CHUNKS = C // 128                  # 16 contraction chunks
Q = 4                                # quarters per core
TOK_Q = TOK_PER_CORE // Q            # 512 tokens per quarter
BLK_Q = TOK_Q // 128                 # 4 blocks of 128 tokens per quarter
G = 4                                # DMA chunk-groups per quarter
CPG = N_CHUNKS // G                  # 4 c-chunks per group
ALPHA = 0.001
ROUTED_SCALING = 1.0

F32 = mybir.dt.float32
U32 = mybir.dt.uint32


def build_nc(mm_dtype=mybir.dt.float32r):
    nc = bacc.Bacc("TRN2", target_bir_lowering=False, debug=False)

    xT = nc.dram_tensor("xT", (C, TOK_PER_CORE), F32, kind="ExternalInput")
    wT = nc.dram_tensor("wT", (C, E), F32, kind="ExternalInput")
    idx8 = nc.dram_tensor("idx8", (Q, BLK_Q, 128, 8), U32, kind="ExternalOutput")
    val8 = nc.dram_tensor("val8", (Q, BLK_Q, 128, 8), F32, kind="ExternalOutput")
    csum = nc.dram_tensor("csum", (Q, 128, E), F32, kind="ExternalOutput")
    ssum = nc.dram_tensor("ssum", (Q, 128, E), F32, kind="ExternalOutput")

    # (c, t) -> (p, k, t): chunk k, partition p = row k*128+p
    xT_v = xT.ap().rearrange("(k p) t -> p k t", p=128)
    wT_v = wT.ap().rearrange("(k p) e -> p k e", p=128)

    with tile.TileContext(nc) as tc, ExitStack() as ctx:
        const_pool = ctx.enter_context(tc.tile_pool(name="const", bufs=1))
        xpool = ctx.enter_context(tc.tile_pool(name="x", bufs=2))
        ps_mm = ctx.enter_context(tc.tile_pool(name="ps_mm", bufs=2, space="PSUM"))
        ps_tr = ctx.enter_context(tc.tile_pool(name="ps_tr", bufs=2, space="PSUM"))
        work = ctx.enter_context(tc.tile_pool(name="work", bufs=2))
        outp = ctx.enter_context(tc.tile_pool(name="outp", bufs=2))

        ident = const_pool.tile([128, 128], F32)
        masks.make_identity(nc, ident[:])

        # weight: chunk 0 first (tiny, unblocks the first matmul), rest after
        wt = const_pool.tile([128, N_CHUNKS, E], F32)
        nc.sync.dma_start(wt[:, 0, :], wT_v[:, 0, :])
        nc.sync.dma_start(wt[:, 1:, :], wT_v[:, 1:, :])

        for q in range(Q):
            tq = slice(q * TOK_Q, (q + 1) * TOK_Q)
            xg = []
            if q == 0:
                # fine-grained chunk DMAs so the PE starts after ~256KB
                for k in range(N_CHUNKS):
                    t = xpool.tile([128, 1, TOK_Q], F32, tag=f"x0_{k}")
                    nc.sync.dma_start(t[:], xT_v[:, k:k + 1, tq])
                    xg.append((t, 0))
            else:
                gts = []
                for g in range(G):
                    t = xpool.tile([128, CPG, TOK_Q], F32, tag=f"xg{g}")
                    nc.sync.dma_start(t[:], xT_v[:, g * CPG:(g + 1) * CPG, tq])
                    gts.append(t)
                xg = [(gts[k // CPG], k % CPG) for k in range(N_CHUNKS)]

            # logitsT (64, 512) accumulated over 16 c-chunks
            ps = ps_mm.tile([64, TOK_Q], F32)
            for k in range(N_CHUNKS):
                t, kk = xg[k]
                nc.tensor.matmul(
                    ps[:],
                    wt[:, k, :].bitcast(mm_dtype),
                    t[:, kk, :].bitcast(mm_dtype),
                    start=(k == 0),
                    stop=(k == N_CHUNKS - 1),
                )

            lt = work.tile([64, TOK_Q], F32, tag="lt")
            nc.vector.tensor_copy(lt[:], ps[:])

            # transpose to (128 tokens, 64 experts) per 128-token block
            pt = ps_tr.tile([128, BLK_Q, E], F32)
            for j in range(BLK_Q):
                nc.tensor.transpose(
                    pt[:, j, :], lt[:, j * 128:(j + 1) * 128], ident[:64, :64]
                )
            lg = work.tile([128, BLK_Q, E], F32, tag="lg")
            nc.vector.tensor_copy(lg[:], pt[:])

            # top-8 (desc) values + indices per token
            mx = work.tile([128, BLK_Q, 8], F32, tag="mx")
            ix = work.tile([128, BLK_Q, 8], U32, tag="ix")
            for j in range(BLK_Q):
                nc.vector.max(mx[:, j, :], lg[:, j, :])
                nc.vector.max_index(ix[:, j, :], mx[:, j, :], lg[:, j, :])

            # softmax pieces (no max subtraction; logits are O(1))
            eg = work.tile([128, BLK_Q, E], F32, tag="eg")
            nc.scalar.activation(eg[:], lg[:], mybir.ActivationFunctionType.Exp)
            dn = work.tile([128, BLK_Q], F32, tag="dn")
            nc.vector.reduce_sum(dn[:], eg[:], axis=mybir.AxisListType.X)
            rc = work.tile([128, BLK_Q], F32, tag="rc")
            nc.vector.reciprocal(rc[:], dn[:])

            e8 = work.tile([128, BLK_Q, 8], F32, tag="e8")
            nc.scalar.activation(e8[:], mx[:], mybir.ActivationFunctionType.Exp)
            w8 = outp.tile([128, BLK_Q, 8], F32, tag="w8")
            nc.vector.tensor_tensor(
                w8[:], e8[:], rc[:].unsqueeze(-1).broadcast_to((128, BLK_Q, 8)),
                op=mybir.AluOpType.mult,
            )

            # full scores + per-expert partial sums (over this quarter's tokens)
            sc = work.tile([128, BLK_Q, E], F32, tag="sc")
            nc.vector.tensor_tensor(
                sc[:], eg[:], rc[:].unsqueeze(-1).broadcast_to((128, BLK_Q, E)),
                op=mybir.AluOpType.mult,
            )
            sv = outp.tile([128, E], F32, tag="sv")
            nc.vector.reduce_sum(
                sv[:], sc[:].transpose((0, 2, 1)), axis=mybir.AxisListType.X
            )

            # count mask: logit >= 6th-largest  (exactly top-6 barring exact ties)
            mk = work.tile([128, BLK_Q, E], F32, tag="mk")
            nc.vector.tensor_tensor(
                mk[:], lg[:], mx[:, :, TOP_K - 1].unsqueeze(-1).broadcast_to((128, BLK_Q, E)),
                op=mybir.AluOpType.is_ge,
            )
            cv = outp.tile([128, E], F32, tag="cv")
            nc.vector.reduce_sum(
                cv[:], mk[:].transpose((0, 2, 1)), axis=mybir.AxisListType.X
            )

            # outputs: SBUF (p, j, k) -> DRAM (q, j, p, k); scalar-engine HWDGE
            # ring so they don't queue behind input DMAs on the sync ring
            nc.scalar.dma_start(idx8.ap()[q].transpose((1, 0, 2)), ix[:])
            nc.scalar.dma_start(val8.ap()[q].transpose((1, 0, 2)), w8[:])
            nc.scalar.dma_start(csum.ap()[q], cv[:])
            nc.scalar.dma_start(ssum.ap()[q], sv[:])

    nc.compile()
    return nc, (xT, wT, idx8, val8, csum, ssum)


_NC_CACHE = {}


def _get_nc(mm_dtype_name):
    if mm_dtype_name not in _NC_CACHE:
        _NC_CACHE[mm_dtype_name] = build_nc(getattr(mybir.dt, mm_dtype_name))
    return _NC_CACHE[mm_dtype_name]


def _postprocess(results):
    """Combine per-core outputs into full (topk_idx, topk_weight, aux_loss)."""
    idx_parts, w_parts = [], []
    counts = np.zeros((N_CORES, E), np.float32)
    scores = np.zeros((N_CORES, E), np.float32)
    for c, out in enumerate(results):
        i8 = out["idx8"]          # (Q, BLK_Q, 128, 8)
        v8 = out["val8"]
        idx_parts.append(i8.reshape(TOK_PER_CORE, 8)[:, :TOP_K].astype(np.int32))
        w_parts.append(v8.reshape(TOK_PER_CORE, 8)[:, :TOP_K].astype(np.float32))
        counts[c] = out["csum"].sum(axis=(0, 1))
        scores[c] = out["ssum"].sum(axis=(0, 1))
    topk_idx = np.concatenate(idx_parts, axis=0)
    topk_weight = np.concatenate(w_parts, axis=0) * np.float32(ROUTED_SCALING)

    # aux loss: combine the two shards of each batch row
    ce = counts.reshape(B, 2, E).sum(axis=1) / (T * TOP_K / E)
    mean_scores = scores.reshape(B, 2, E).sum(axis=1) / T
    aux_loss = np.float32((ce * mean_scores).sum(axis=1).mean() * ALPHA)
    return topk_idx, topk_weight, aux_loss


def _make_in_maps(x, weight):
    xf = np.ascontiguousarray(np.asarray(x, dtype=np.float32).reshape(N_TOKENS, C))
    wT = np.ascontiguousarray(np.asarray(weight, dtype=np.float32).T)
    in_maps = []
    for c in range(N_CORES):
        shard = np.ascontiguousarray(xf[c * TOK_PER_CORE:(c + 1) * TOK_PER_CORE].T)
        in_maps.append({"xT": shard, "wT": wT})
    return in_maps


def kernel(x, weight, mm_dtype_name="float32r", trace=False):
    nc, _ = _get_nc(mm_dtype_name)
    in_maps = _make_in_maps(x, weight)
    res = bass_utils.run_bass_kernel_spmd(
        nc, in_maps, core_ids=list(range(N_CORES)), trace=trace
    )
    out = _postprocess(res.results)
    if trace:
        return out, res
    return out


# revision 10
# speedup vs baseline: 1.1129x; 1.0900x over previous
"""MoE gate (softmax routing, top-6 of 64 experts) for Trainium2, 8 NeuronCores.

Problem: x (4, 4096, 2048) f32, gate weight (64, 2048) f32.
  logits = x @ w.T          (16384, 64)
  scores = softmax(logits)
  topk_weight, topk_idx = top_k(scores, 6)       (sorted desc)
  aux_loss = seq-aux load-balancing loss (scalar)

Sharding: data-parallel over the flattened token dim — 2048 tokens per core.
Each core's shard is fed pre-transposed (C-major) so the contraction dim C
lands on SBUF partitions for the PE matmul. The gate weight (tiny) is
replicated. Per-core partial count/score sums are combined on the host into
the scalar aux loss.

Device pipeline per core (tokens processed in quarters of 512):
  DMA xT chunk-groups -> PE matmul (wT stationary, tokens moving) accumulating
  logitsT (64, 512) in PSUM over 16 c-chunks -> copy to SBUF -> PE transpose
  to (128 tok, 64 e) -> DVE max8/max_index (top-8 per token, desc order) ->
  ACT exp (no max-subtract: |logits| is O(1)) -> DVE reduce/reciprocal for
  softmax denom -> top-k weights, per-expert count + score partial sums.
"""

import os
import sys
import numpy as np
from contextlib import ExitStack

sys.path.insert(0, "/opt/trn_rl_repo")

import concourse.bass as bass
import concourse.bacc as bacc
import concourse.mybir as mybir
import concourse.tile as tile
from concourse import masks
from concourse import bass_utils

# ---- problem constants (hardcoded per the contract) ----
TOP_K = 6
E = 64               # experts
C = 2048             # feature dim
B, T = 4, 4096
N_TOKENS = B * T     # 16384
N_CORES = 8
TOK_PER_CORE = N_TOKENS // N_CORES   # 2048
N_CHUNKS = C // 128                  # 16 contraction chunks
Q = 4                                # quarters per core
TOK_Q = TOK_PER_CORE // Q            # 512 tokens per quarter
BLK_Q = TOK_Q // 128                 # 4 blocks of 128 tokens per quarter
G = 4                                # DMA chunk-groups per quarter
CPG = N_CHUNKS // G                  # 4 c-chunks per group
ALPHA = 0.001
ROUTED_SCALING = 1.0

F32 = mybir.dt.float32
U32 = mybir.dt.uint32


def build_nc(mm_dtype=mybir.dt.float32, paired=True):
    nc = bacc.Bacc("TRN2", target_bir_lowering=False, debug=False)

    # For float32r matmuls, declare x/w as float32r end-to-end (same bits as
    # f32; the PE reads reduced mantissa). The verifier requires f32r matmul
    # inputs to have f32r provenance.
    DT_X = mm_dtype

    xT = nc.dram_tensor("xT", (C, TOK_PER_CORE), DT_X, kind="ExternalInput")
    wT = nc.dram_tensor("wT", (C, E), DT_X, kind="ExternalInput")
    idx8 = nc.dram_tensor("idx8", (Q, BLK_Q, 128, 8), U32, kind="ExternalOutput")
    val8 = nc.dram_tensor("val8", (Q, BLK_Q, 128, 8), F32, kind="ExternalOutput")
    csum = nc.dram_tensor("csum", (Q, 128, E), F32, kind="ExternalOutput")
    ssum = nc.dram_tensor("ssum", (Q, 128, E), F32, kind="ExternalOutput")

    # (c, t) -> (p, k, t): chunk k, partition p = row k*128+p
    xT_v = xT.ap().rearrange("(k p) t -> p k t", p=128)
    wT_v = wT.ap().rearrange("(k p) e -> p k e", p=128)

    with tile.TileContext(nc) as tc, ExitStack() as ctx:
        const_pool = ctx.enter_context(tc.tile_pool(name="const", bufs=1))
        xpool = ctx.enter_context(tc.tile_pool(name="x", bufs=2))
        ps_mm = ctx.enter_context(tc.tile_pool(name="ps_mm", bufs=2, space="PSUM"))
        ps_tr = ctx.enter_context(tc.tile_pool(name="ps_tr", bufs=2, space="PSUM"))
        work = ctx.enter_context(tc.tile_pool(name="work", bufs=2))
        outp = ctx.enter_context(tc.tile_pool(name="outp", bufs=2))

        ident = const_pool.tile([128, 128], F32)
        masks.make_identity(nc, ident[:])

        # weight: chunk 0 first (tiny, unblocks the first matmul), rest after
        wt = const_pool.tile([128, N_CHUNKS, E], DT_X)
        nc.sync.dma_start(wt[:, 0, :], wT_v[:, 0, :])
        nc.sync.dma_start(wt[:, 1:, :], wT_v[:, 1:, :])

        for q in range(Q):
            tq = slice(q * TOK_Q, (q + 1) * TOK_Q)
            xg = []
            if q == 0:
                # fine-grained chunk DMAs so the PE starts after ~256KB
                for k in range(N_CHUNKS):
                    t = xpool.tile([128, 1, TOK_Q], DT_X, tag=f"x0_{k}")
                    nc.sync.dma_start(t[:], xT_v[:, k:k + 1, tq])
                    xg.append((t, 0))
            else:
                gts = []
                for g in range(G):
                    t = xpool.tile([128, CPG, TOK_Q], DT_X, tag=f"xg{g}")
                    nc.sync.dma_start(t[:], xT_v[:, g * CPG:(g + 1) * CPG, tq])
                    gts.append(t)
                xg = [(gts[k // CPG], k % CPG) for k in range(N_CHUNKS)]

            # logitsT (64, 512) accumulated over 16 c-chunks
            lt = work.tile([64, TOK_Q], F32, tag="lt")
            if paired:
                # pack chunk pairs onto disjoint PE column halves so two
                # matmuls stream concurrently; PSUM top half = even chunks,
                # bottom half = odd chunks; summed in the PSUM->SBUF pass
                ps = ps_mm.tile([128, TOK_Q], F32)
                for kp in range(N_CHUNKS // 2):
                    ta, ka = xg[2 * kp]
                    tb, kb = xg[2 * kp + 1]
                    nc.tensor.matmul(
                        ps[0:64, :], wt[:, 2 * kp, :], ta[:, ka, :],
                        start=(kp == 0), stop=(kp == N_CHUNKS // 2 - 1),
                        tile_position=(0, 0), skip_group_check=True,
                    )
                    nc.tensor.matmul(
                        ps[64:128, :], wt[:, 2 * kp + 1, :], tb[:, kb, :],
                        start=(kp == 0), stop=(kp == N_CHUNKS // 2 - 1),
                        tile_position=(0, 64), skip_group_check=True,
                    )
                nc.vector.tensor_copy(lt[:], ps[0:64, :])
                nc.vector.tensor_tensor(
                    lt[:], lt[:], ps[64:128, :], op=mybir.AluOpType.add
                )
            else:
                ps = ps_mm.tile([64, TOK_Q], F32)
                for k in range(N_CHUNKS):
                    t, kk = xg[k]
                    nc.tensor.matmul(
                        ps[:],
                        wt[:, k, :],
                        t[:, kk, :],
                        start=(k == 0),
                        stop=(k == N_CHUNKS - 1),
                    )
                nc.vector.tensor_copy(lt[:], ps[:])

            # transpose to (128 tokens, 64 experts) per 128-token block
            pt = ps_tr.tile([128, BLK_Q, E], F32)
            for j in range(BLK_Q):
                nc.tensor.transpose(
                    pt[:, j, :], lt[:, j * 128:(j + 1) * 128], ident[:64, :64]
                )
            lg = work.tile([128, BLK_Q, E], F32, tag="lg")
            nc.vector.tensor_copy(lg[:], pt[:])

            # top-8 (desc) values + indices per token
            mx = work.tile([128, BLK_Q, 8], F32, tag="mx")
            ix = work.tile([128, BLK_Q, 8], U32, tag="ix")
            for j in range(BLK_Q):
                nc.vector.max(mx[:, j, :], lg[:, j, :])
                nc.vector.max_index(ix[:, j, :], mx[:, j, :], lg[:, j, :])

            # softmax pieces (no max subtraction; logits are O(1))
            eg = work.tile([128, BLK_Q, E], F32, tag="eg")
            nc.scalar.activation(eg[:], lg[:], mybir.ActivationFunctionType.Exp)
            dn = work.tile([128, BLK_Q], F32, tag="dn")
            nc.vector.reduce_sum(dn[:], eg[:], axis=mybir.AxisListType.X)
            rc = work.tile([128, BLK_Q], F32, tag="rc")
            nc.vector.reciprocal(rc[:], dn[:])

            e8 = work.tile([128, BLK_Q, 8], F32, tag="e8")
            nc.scalar.activation(e8[:], mx[:], mybir.ActivationFunctionType.Exp)
            w8 = outp.tile([128, BLK_Q, 8], F32, tag="w8")
            nc.vector.tensor_tensor(
                w8[:], e8[:], rc[:].unsqueeze(-1).broadcast_to((128, BLK_Q, 8)),
                op=mybir.AluOpType.mult,
            )

            # full scores + per-expert partial sums (over this quarter's tokens)
            sc = work.tile([128, BLK_Q, E], F32, tag="sc")
            nc.vector.tensor_tensor(
                sc[:], eg[:], rc[:].unsqueeze(-1).broadcast_to((128, BLK_Q, E)),
                op=mybir.AluOpType.mult,
            )
            sv = outp.tile([128, E], F32, tag="sv")
            nc.vector.reduce_sum(
                sv[:], sc[:].transpose((0, 2, 1)), axis=mybir.AxisListType.X
            )

            # count mask: logit >= 6th-largest  (exactly top-6 barring exact ties)
            mk = work.tile([128, BLK_Q, E], F32, tag="mk")
            nc.vector.tensor_tensor(
                mk[:], lg[:], mx[:, :, TOP_K - 1].unsqueeze(-1).broadcast_to((128, BLK_Q, E)),
                op=mybir.AluOpType.is_ge,
            )
            cv = outp.tile([128, E], F32, tag="cv")
            nc.vector.reduce_sum(
                cv[:], mk[:].transpose((0, 2, 1)), axis=mybir.AxisListType.X
            )

            # outputs: SBUF (p, j, k) -> DRAM (q, j, p, k); scalar-engine HWDGE
            # ring so they don't queue behind input DMAs on the sync ring
            nc.scalar.dma_start(idx8.ap()[q].transpose((1, 0, 2)), ix[:])
            nc.scalar.dma_start(val8.ap()[q].transpose((1, 0, 2)), w8[:])
            nc.scalar.dma_start(csum.ap()[q], cv[:])
            nc.scalar.dma_start(ssum.ap()[q], sv[:])

    nc.compile()
    return nc, (xT, wT, idx8, val8, csum, ssum)


_NC_CACHE = {}


def _get_nc(mm_dtype_name, paired=True):
    key = (mm_dtype_name, paired)
    if key not in _NC_CACHE:
        _NC_CACHE[key] = build_nc(getattr(mybir.dt, mm_dtype_name), paired=paired)
    return _NC_CACHE[key]


def _postprocess(results):
    """Combine per-core outputs into full (topk_idx, topk_weight, aux_loss)."""
    idx_parts, w_parts = [], []
    counts = np.zeros((N_CORES, E), np.float32)
    scores = np.zeros((N_CORES, E), np.float32)
    for c, out in enumerate(results):
        i8 = out["idx8"]          # (Q, BLK_Q, 128, 8)
        v8 = out["val8"]
        idx_parts.append(i8.reshape(TOK_PER_CORE, 8)[:, :TOP_K].astype(np.int32))
        w_parts.append(v8.reshape(TOK_PER_CORE, 8)[:, :TOP_K].astype(np.float32))
        counts[c] = out["csum"].sum(axis=(0, 1))
        scores[c] = out["ssum"].sum(axis=(0, 1))
    topk_idx = np.concatenate(idx_parts, axis=0)
    topk_weight = np.concatenate(w_parts, axis=0) * np.float32(ROUTED_SCALING)

    # aux loss: combine the two shards of each batch row
    ce = counts.reshape(B, 2, E).sum(axis=1) / (T * TOP_K / E)
    mean_scores = scores.reshape(B, 2, E).sum(axis=1) / T
    aux_loss = np.float32((ce * mean_scores).sum(axis=1).mean() * ALPHA)
    return topk_idx, topk_weight, aux_loss


def _make_in_maps(x, weight):
    xf = np.ascontiguousarray(np.asarray(x, dtype=np.float32).reshape(N_TOKENS, C))
    wT = np.ascontiguousarray(np.asarray(weight, dtype=np.float32).T)
    in_maps = []
    for c in range(N_CORES):
        shard = np.ascontiguousarray(xf[c * TOK_PER_CORE:(c + 1) * TOK_PER_CORE].T)
        in_maps.append({"xT": shard, "wT": wT})
    return in_maps


def kernel(x, weight, mm_dtype_name="float32", paired=True, trace=False):
    nc, _ = _get_nc(mm_dtype_name, paired)
    in_maps = _make_in_maps(x, weight)
    res = bass_utils.run_bass_kernel_spmd(
        nc, in_maps, core_ids=list(range(N_CORES)), trace=trace
    )
    out = _postprocess(res.results)
    if trace:
        return out, res
    return out
